# revision 19
# baseline (speedup 1.0000x reference)
"""Sharded KNN retrieval kernel for Trainium2 (8 NeuronCores).

Problem: 2048 one-hot-encoded query utterances vs 100k one-hot-encoded
support utterances; top-1 nearest neighbor by squared L2, first-index
tie-breaking; output = one-hot of the winner's meanings row.

Because both sides are one-hot, squared distance reduces to
    dist(n, s) = const - 2 * match_count(n, s),   match_count in [0, 16]
so argmin(dist) = argmax(match_count) with first-index tie-break. All
arithmetic is small integers, exact in fp8/fp16/fp32, so value+index are
encoded in the matmul output itself:

  support sharded 12500 rows/core = 25 blocks x 500 columns (no padding).
  TensorE:  psum[nt, b] = bfT.T @ (-32 * supT)      (= -32*match_count)
            fp8e4 DoubleRow, query-tile-outer loop so the stationary
            operand is reused across each 8-block group (few LDWEIGHTS).
  Drains:   split across ScalarE + VectorE to balance engine load
            (VectorE alone is the baseline bottleneck):
              b == 0:        ACT copy psum -> run[nt] (fp16, init)
              b%5 in {1,3}:  DVE  run = min(psum + b, run)   (fp32 path)
              else:          ACT val = psum + b (fp16); DVE run = min(val,
                             run) at fp16 2x throughput
  Final:    key = run*500 + j  (= -16000*match + s_local, exact fp32),
            min-reduce over j -> out[:, nt]; overlaps next nt's matmuls.
  Host:     per-core decode (match, s_local), global lexicographic min
            over cores by (match desc, global index asc), gather meanings,
            one-hot. Everything is exact integer arithmetic.
"""

import sys
import time

import numpy as np

if "/opt/trn_rl_repo" not in sys.path:
    sys.path.insert(0, "/opt/trn_rl_repo")

import ml_dtypes

VOCAB = 32
UTT_LEN = 16
K_DIM = VOCAB * UTT_LEN  # 512
N_QUERIES = 2048
S_FULL = 100000
N_CORES = 8
S_SHARD = S_FULL // N_CORES  # 12500
BLOCK = 500
N_BLOCKS = S_SHARD // BLOCK  # 25
N_TILES = N_QUERIES // 128  # 16
N_STRIPS = 5
STRIP_W = S_SHARD // N_STRIPS  # 2500
GROUP = 8  # psum banks per matmul phase group
MEANINGS_PER_TYPE = 10
SCALE = 32.0  # support multiplier; needs 2^5 > N_BLOCKS
ENC = int(SCALE) * BLOCK  # 16000: key = -16000*match + s_local

DEFAULT_VARIANT = "zz9"

_CACHE = {}
LAST_RESULTS = None  # BassKernelResults of the most recent device run
LAST_WALL_NS = None


def _build_bass(reps=1, variant="full"):
    import concourse.bacc as bacc
    import concourse.tile as tile
    from concourse import mybir

    nc = bacc.Bacc(
        "TRN2", target_bir_lowering=False, debug=False, enable_asserts=False
    )
    fp8e4 = mybir.dt.float8e4
    f16 = mybir.dt.float16
    f32 = mybir.dt.float32

    # drain-engine assignment: block 0 initializes run via ACT; "direct"
    # blocks go on DVE (fp32 STT from psum); the rest via ACT copy + DVE
    # fp16 min. The split balances ACT vs DVE load under the TensorE
    # roofline. zigzag alternates the accumulation-phase order per group so
    # adjacent groups share the stationary operand (fewer LDWEIGHTS).
    group = GROUP
    if variant.startswith("g4"):
        group = 4
        variant = variant[2:]
    base = variant[2:] if variant.startswith("zz") else variant
    zigzag = variant.startswith("zz")
    unroll2 = base.endswith("u2")
    if unroll2:
        base = base[:-2]
    prefetch_lw = base.endswith("p")
    if prefetch_lw:
        base = base[:-1]
    gp_finals = base.endswith("f")
    if gp_finals:
        base = base[:-1]
    pair_drain = base == "pd"
    two_chain = base == "c2"
    if two_chain:
        # 12 direct blocks, 6 per chain, alternating with ACT-path blocks
        direct = {3, 4, 7, 8, 11, 12, 15, 16, 19, 20, 23, 24}
    elif base == "9":
        direct = {1, 3, 6, 8, 11, 13, 16, 18, 21}
    elif base == "6":
        direct = {3, 7, 11, 15, 19, 23}
    else:
        direct = {b for b in range(N_BLOCKS) if b % 5 in (1, 3)}

    # DoubleRow layout: K=512 split as 2 groups x (2 k-tiles x 128)
    supT = nc.dram_tensor(
        "supT", [2, 128, 2, S_SHARD], fp8e4, kind="ExternalInput"
    ).ap()
    bfT = nc.dram_tensor(
        "bfT", [2, 128, 2, N_QUERIES], fp8e4, kind="ExternalInput"
    ).ap()
    if pair_drain:
        jrow = nc.dram_tensor(
            "jrow", [128, 2, BLOCK], f32, kind="ExternalInput"
        ).ap()
    else:
        jrow = nc.dram_tensor(
            "jrow", [128, BLOCK], f32, kind="ExternalInput"
        ).ap()
    out = nc.dram_tensor("out", [128, N_TILES], f32, kind="ExternalOutput").ap()

    with tile.TileContext(nc) as tc:
        with (
            tc.tile_pool(name="sup", bufs=1) as sup_pool,
            tc.tile_pool(name="bq", bufs=1) as bq_pool,
            tc.tile_pool(
                name="ps", bufs=4 if pair_drain else 8, space="PSUM"
            ) as ps_pool,
            tc.tile_pool(name="run", bufs=1) as run_pool,
            tc.tile_pool(name="fin", bufs=1) as fin_pool,
            tc.tile_pool(
                name="val", bufs=4 if pair_drain else (5 if two_chain else 6)
            ) as val_pool,
        ):

            def body(h=""):
                # -- input DMAs, spread over two idle engine queues (sync,
                # pool) and ordered so WAR deps resolve early across reps.
                bq_tiles = []
                for c in range(2):
                    t = bq_pool.tile(
                        [128, 2, N_QUERIES], fp8e4, tag=f"bq{c}{h}", name=f"bq{c}{h}"
                    )
                    nc.sync.dma_start(t[:], bfT[c])
                    bq_tiles.append(t)
                sup_tiles = {}
                for q in range(N_STRIPS):
                    for c in range(2):
                        t = sup_pool.tile(
                            [128, 2, STRIP_W], fp8e4,
                            tag=f"sup{c}_{q}{h}", name=f"sup{c}_{q}{h}",
                        )
                        eng = nc.sync if c == 0 else nc.gpsimd
                        eng.dma_start(
                            t[:], supT[c, :, :, STRIP_W * q : STRIP_W * (q + 1)]
                        )
                        sup_tiles[(c, q)] = t
                # second copy of the queries, used only by the last query
                # tile: lets the main copy's re-DMA overlap nt=15 compute.
                bq2_tiles = []
                if not pair_drain:
                    for c in range(2):
                        t = bq_pool.tile(
                            [128, 2, N_QUERIES], fp8e4,
                            tag=f"bq2_{c}{h}", name=f"bq2_{c}{h}",
                        )
                        nc.gpsimd.dma_start(t[:], bfT[c])
                        bq2_tiles.append(t)
                jrow_t = fin_pool.tile(
                    [128, 2, BLOCK] if pair_drain else [128, BLOCK],
                    f32, tag=f"jrow{h}", name=f"jrow_t{h}",
                )
                nc.gpsimd.dma_start(jrow_t[:], jrow[:])

                run_tiles = [
                    run_pool.tile(
                        [128, 2, BLOCK] if pair_drain else [128, BLOCK],
                        f16, tag=f"run{h}_{nt}", name=f"run{h}_{nt}",
                    )
                    for nt in range(N_TILES)
                ]
                runb_tiles = [
                    run_pool.tile(
                        [128, BLOCK], f16, tag=f"runb{h}_{nt}", name=f"runb{h}_{nt}"
                    )
                    for nt in range(N_TILES)
                ] if two_chain else None
                fin = fin_pool.tile(
                    [128, N_TILES], f32, tag=f"fin{h}", name=f"fin{h}"
                )
                if variant == "nodrain":
                    nc.gpsimd.memset(fin[:], 0.0)

                for nt in range(N_TILES):
                    bq_use = (
                        bq2_tiles
                        if (nt == N_TILES - 1 and not pair_drain)
                        else bq_tiles
                    )
                    run = run_tiles[nt]
                    if pair_drain:
                        # 12 bank-pairs + 1 single block; one strided drain
                        # covers both banks of a pair (halves drain op count)
                        units = [(2 * p, 2 * p + 1) for p in range(12)]
                        units.append((24,))
                        pd_groups = [
                            units[0:4], units[4:8], units[8:12], units[12:13]
                        ]
                        pd_direct = {2, 5, 8, 11}
                        for gi, gunits in enumerate(pd_groups):
                            ps_t = {
                                u: ps_pool.tile(
                                    [128, 2, 512], f32, tag="ps",
                                    name=f"ps{h}_{nt}_{u[0]}",
                                )
                                for u in gunits
                            }
                            phases = (0, 1)
                            if zigzag and gi % 2 == 1:
                                phases = (1, 0)
                            for pi, c in enumerate(phases):
                                lhsT = bq_use[c][:, :, 128 * nt : 128 * (nt + 1)]
                                for u in gunits:
                                    for k, b in enumerate(u):
                                        q, rem = divmod(BLOCK * b, STRIP_W)
                                        nc.tensor.matmul(
                                            ps_t[u][:, k, 0:BLOCK],
                                            lhsT,
                                            sup_tiles[(c, q)][
                                                :, :, rem : rem + BLOCK
                                            ],
                                            start=(pi == 0),
                                            stop=(pi == 1),
                                            perf_mode=mybir.MatmulPerfMode.DoubleRow,
                                        )
                            for u in gunits:
                                p = u[0] // 2
                                pair = len(u) == 2
                                src_ap = (
                                    ps_t[u][:, :, 0:BLOCK]
                                    if pair
                                    else ps_t[u][:, 0, 0:BLOCK]
                                )
                                run_ap = run[:, :, 0:BLOCK] if pair else run[:, 0, 0:BLOCK]
                                if p == 0:
                                    nc.scalar.activation(
                                        out=run_ap,
                                        in_=src_ap,
                                        func=mybir.ActivationFunctionType.Copy,
                                        bias=0.0,
                                        scale=1.0,
                                    )
                                elif p in pd_direct:
                                    nc.vector.scalar_tensor_tensor(
                                        out=run_ap,
                                        in0=src_ap,
                                        scalar=float(p),
                                        in1=run_ap,
                                        op0=mybir.AluOpType.add,
                                        op1=mybir.AluOpType.min,
                                    )
                                else:
                                    val = val_pool.tile(
                                        [128, 2, BLOCK], f16, tag="val",
                                        name=f"val{h}_{nt}_{p}",
                                    )
                                    val_ap = (
                                        val[:, :, 0:BLOCK]
                                        if pair
                                        else val[:, 0, 0:BLOCK]
                                    )
                                    nc.scalar.activation(
                                        out=val_ap,
                                        in_=src_ap,
                                        func=mybir.ActivationFunctionType.Copy,
                                        bias=float(p),
                                        scale=1.0,
                                    )
                                    nc.vector.tensor_tensor(
                                        run_ap, val_ap, run_ap,
                                        mybir.AluOpType.min,
                                    )
                        # key = run*1000 + j == -32000*match + s_local, exact
                        key = val_pool.tile(
                            [128, 2, BLOCK], f32, tag="key", bufs=2,
                            name=f"key{h}_{nt}",
                        )
                        nc.vector.scalar_tensor_tensor(
                            out=key[:],
                            in0=run[:],
                            scalar=float(2 * BLOCK),
                            in1=jrow_t[:],
                            op0=mybir.AluOpType.mult,
                            op1=mybir.AluOpType.add,
                        )
                        nc.vector.tensor_reduce(
                            out=fin[:, nt : nt + 1],
                            in_=key[:],
                            axis=mybir.AxisListType.XY,
                            op=mybir.AluOpType.min,
                        )
                        continue
                    for gi, g0 in enumerate(range(0, N_BLOCKS, group)):
                        blocks = range(g0, min(g0 + group, N_BLOCKS))
                        ps_t = {
                            b: ps_pool.tile(
                                [128, BLOCK], f32, tag="ps", name=f"ps{h}_{nt}_{b}"
                            )
                            for b in blocks
                        }
                        phases = (0, 1)
                        if zigzag and gi % 2 == 1:
                            phases = (1, 0)
                        for pi, c in enumerate(phases):
                            # same stationary operand for the whole phase
                            lhsT = bq_use[c][:, :, 128 * nt : 128 * (nt + 1)]
                            for b in blocks:
                                q, rem = divmod(BLOCK * b, STRIP_W)
                                nc.tensor.matmul(
                                    ps_t[b][:],
                                    lhsT,
                                    sup_tiles[(c, q)][:, :, rem : rem + BLOCK],
                                    start=(pi == 0),
                                    stop=(pi == 1),
                                    perf_mode=mybir.MatmulPerfMode.DoubleRow,
                                )
                        if base == "nodrain":
                            continue
                        for b in blocks:
                            if two_chain:
                                run = (
                                    run_tiles[nt]
                                    if (b < 2 and b == 0) or (b >= 2 and b % 2 == 0)
                                    else runb_tiles[nt]
                                )
                            if b == 0 or (two_chain and b == 1):
                                # initializes the chain; no memset needed
                                nc.scalar.activation(
                                    out=run[:],
                                    in_=ps_t[b][:],
                                    func=mybir.ActivationFunctionType.Copy,
                                    bias=float(b),
                                    scale=1.0,
                                )
                            elif b in direct:
                                # run = min(psum + b, run): fp32 psum path
                                nc.vector.scalar_tensor_tensor(
                                    out=run[:],
                                    in0=ps_t[b][:],
                                    scalar=float(b),
                                    in1=run[:],
                                    op0=mybir.AluOpType.add,
                                    op1=mybir.AluOpType.min,
                                )
                            else:
                                val = val_pool.tile(
                                    [128, BLOCK], f16, tag="val",
                                    name=f"val{h}_{nt}_{b}",
                                )
                                nc.scalar.activation(
                                    out=val[:],
                                    in_=ps_t[b][:],
                                    func=mybir.ActivationFunctionType.Copy,
                                    bias=float(b),
                                    scale=1.0,
                                )
                                nc.vector.tensor_tensor(
                                    run[:], val[:], run[:], mybir.AluOpType.min
                                )
                    if variant == "nodrain":
                        continue
                    if two_chain:
                        merge_eng = nc.gpsimd if gp_finals else nc.vector
                        merge_eng.tensor_tensor(
                            run_tiles[nt][:],
                            runb_tiles[nt][:],
                            run_tiles[nt][:],
                            mybir.AluOpType.min,
                        )
                        run = run_tiles[nt]
                    # key = run*500 + j  ==  -16000*match + s_local, exact
                    key = val_pool.tile(
                        [128, BLOCK], f32, tag="key", bufs=2, name=f"key{h}_{nt}"
                    )
                    key_eng = nc.gpsimd if gp_finals else nc.vector
                    key_eng.scalar_tensor_tensor(
                        out=key[:],
                        in0=run[:],
                        scalar=float(BLOCK),
                        in1=jrow_t[:],
                        op0=mybir.AluOpType.mult,
                        op1=mybir.AluOpType.add,
                    )
                    nc.vector.tensor_reduce(
                        out=fin[:, nt : nt + 1],
                        in_=key[:],
                        axis=mybir.AxisListType.X,
                        op=mybir.AluOpType.min,
                    )
                nc.sync.dma_start(out[:], fin[:])

            if reps == 1:
                body()
            elif unroll2:
                assert reps % 2 == 0, "unroll2 needs even reps"
                with tc.For_i(0, reps // 2, 1):
                    body("")
                    body("B")
            else:
                with tc.For_i(0, reps, 1):
                    body()

    nc.compile()
    return nc


def _get_nc(reps=1, variant=None):
    if variant is None:
        variant = DEFAULT_VARIANT
    key = ("nc", reps, variant)
    if key not in _CACHE:
        _CACHE[key] = _build_bass(reps, variant)
    return _CACHE[key]


def _make_timed_runner(nc, in_maps):
    """Replicates bass2jax.run_bass_via_pjrt's sharded call, but with
    device-resident inputs so repeated invocations time dispatch+execute
    only (no host->device transfer of the 100MB+ of inputs)."""
    import jax
    from jax.sharding import Mesh, NamedSharding, PartitionSpec

    from jax.experimental.shard_map import shard_map

    from concourse import bass2jax, mybir
    from concourse.bass2jax import _bass_exec_p, install_neuronx_cc_hook

    install_neuronx_cc_hook()
    partition_name = (
        nc.partition_id_tensor.name if nc.partition_id_tensor else None
    )
    in_names, out_names, out_avals, zero_outs = [], [], [], []
    for alloc in nc.m.functions[0].allocations:
        if not isinstance(alloc, mybir.MemoryLocationSet):
            continue
        name = alloc.memorylocations[0].name
        if alloc.kind == "ExternalInput":
            if name != partition_name:
                in_names.append(name)
        elif alloc.kind == "ExternalOutput":
            out_names.append(name)
            shape = tuple(alloc.tensor_shape)
            dtype = mybir.dt.np(alloc.dtype)
            out_avals.append(jax.core.ShapedArray(shape, dtype))
            zero_outs.append(np.zeros(shape, dtype))
    n_params = len(in_names)
    n_outs = len(out_avals)
    in_names_full = list(in_names) + out_names
    if partition_name is not None:
        in_names_full.append(partition_name)

    def _body(*args):
        operands = list(args)
        if partition_name is not None:
            operands.append(bass2jax.partition_id_tensor())
        return tuple(
            _bass_exec_p.bind(
                *operands,
                out_avals=tuple(out_avals),
                in_names=tuple(in_names_full),
                out_names=tuple(out_names),
                lowering_input_output_aliases=(),
                sim_require_finite=True,
                sim_require_nnan=True,
                nc=nc,
            )
        )

    devices = jax.devices()[:N_CORES]
    mesh = Mesh(np.asarray(devices), ("core",))
    in_specs = (PartitionSpec("core"),) * (n_params + n_outs)
    out_specs = (PartitionSpec("core"),) * len(out_names)
    donate = tuple(range(n_params, n_params + n_outs))
    sharded = jax.jit(
        shard_map(
            _body, mesh=mesh, in_specs=in_specs, out_specs=out_specs,
            check_rep=False,
        ),
        donate_argnums=donate,
        keep_unused=True,
    )
    sh = NamedSharding(mesh, PartitionSpec("core"))
    concat_in = [
        np.concatenate([np.asarray(in_maps[c][nm]) for c in range(N_CORES)], axis=0)
        for nm in in_names
    ]
    dev_in = [jax.device_put(a, sh) for a in concat_in]

    def call():
        zs = [
            jax.device_put(
                np.zeros((N_CORES * z.shape[0], *z.shape[1:]), z.dtype), sh
            )
            for z in zero_outs
        ]
        jax.block_until_ready(zs)
        t0 = time.perf_counter_ns()
        outs = sharded(*dev_in, *zs)
        jax.block_until_ready(outs)
        dt = time.perf_counter_ns() - t0
        return dt, outs

    return call


def measure_hw_exec_ns(in_maps, r1=25, r2=225, tries=8, variant=None):
    """Per-iteration device time of the full kernel body, measured by
    differencing two in-NEFF repetition counts (cancels dispatch/RPC)."""
    times = {}
    if variant is None:
        variant = DEFAULT_VARIANT
    if variant.endswith("u2"):
        r1 += r1 % 2
        r2 += r2 % 2
    for r in (r1, r2):
        call = _make_timed_runner(_get_nc(reps=r, variant=variant), in_maps)
        call()  # warmup/compile
        times[r] = min(call()[0] for _ in range(tries))
    return (times[r2] - times[r1]) / (r2 - r1), times


def _dr_pack(mat_f32, dt):
    """[512, W] -> [2, 128, 2, W] DoubleRow k-tile packing: k = 256*g + 128*ko + ki."""
    w = mat_f32.shape[1]
    return np.ascontiguousarray(
        mat_f32.reshape(2, 2, 128, w).transpose(0, 2, 1, 3)
    ).astype(dt)


def _is_pd(variant=None):
    if variant is None:
        variant = DEFAULT_VARIANT
    return "pd" in variant


def _prep_in_maps(utts_np, support_np, variant=None):
    bf = utts_np.astype(np.int64)[:, None, :] == np.arange(VOCAB, dtype=np.int64)[
        None, :, None
    ]
    bfT = bf.reshape(K_DIM, N_QUERIES).astype(np.float32)
    bfT_in = _dr_pack(bfT, ml_dtypes.float8_e4m3)
    if _is_pd(variant):
        jrow = np.ascontiguousarray(
            np.broadcast_to(
                np.arange(2 * BLOCK, dtype=np.float32).reshape(2, BLOCK),
                (128, 2, BLOCK),
            )
        )
    else:
        jrow = np.ascontiguousarray(
            np.broadcast_to(np.arange(BLOCK, dtype=np.float32), (128, BLOCK))
        )

    in_maps = []
    for c in range(N_CORES):
        shard = support_np[c * S_SHARD : (c + 1) * S_SHARD]  # [12500, 512]
        supT_c = shard.T * (-SCALE)
        supT_in = _dr_pack(supT_c, ml_dtypes.float8_e4m3)
        in_maps.append({"supT": supT_in, "bfT": bfT_in, "jrow": jrow})
    return in_maps


def _one_hot_meanings(meanings_np, idx):
    meanings = np.asarray(meanings_np)[idx]  # [2048, T]
    n, t = meanings.shape
    out = np.zeros((n, t, MEANINGS_PER_TYPE), dtype=np.float32)
    out[np.arange(n)[:, None], np.arange(t)[None, :], meanings.astype(np.int64)] = 1.0
    return out


def _fallback_numpy(utts_np, support_np, meanings_np):
    """Exact reference semantics in fp32 numpy (for unexpected inputs)."""
    u = utts_np.astype(np.int64)
    m, n = u.shape
    bf = (u.T[:, :, None] == np.arange(VOCAB, dtype=np.int64)).astype(np.float32)
    bf = bf.reshape(n, m * VOCAB)
    sup = support_np.astype(np.float32)
    sup_sq = np.sum(sup * sup, axis=1)
    best_val = np.full(n, np.inf, dtype=np.float32)
    best_idx = np.zeros(n, dtype=np.int64)
    ch = 8192
    for s0 in range(0, sup.shape[0], ch):
        blk = sup[s0 : s0 + ch]
        d = sup_sq[s0 : s0 + ch][None, :] - 2.0 * (bf @ blk.T)
        i = np.argmin(d, axis=1)
        v = d[np.arange(n), i]
        upd = v < best_val  # strict: keeps first occurrence
        best_idx[upd] = s0 + i[upd]
        best_val[upd] = v[upd]
    return _one_hot_meanings(meanings_np, best_idx)


def _is_fast_path(utts_np, support_np, meanings_np):
    if utts_np.shape != (UTT_LEN, N_QUERIES):
        return False
    if support_np.shape != (S_FULL, K_DIM):
        return False
    if meanings_np.shape[0] != S_FULL:
        return False
    if utts_np.min() < 0 or utts_np.max() >= VOCAB:
        return False
    # exact encoding requires {0,1}-valued support with constant row norms
    if not np.all((support_np == 0.0) | (support_np == 1.0)):
        return False
    rs = support_np.sum(axis=1)
    if not np.all(rs == rs[0]):
        return False
    return True


def kernel(utts, support, meanings_t, _trace=False, **_trace_kwargs):
    global LAST_RESULTS, LAST_WALL_NS
    utts_np = np.asarray(utts)
    support_np = np.asarray(support, dtype=np.float32)
    meanings_np = np.asarray(meanings_t)

    if not _is_fast_path(utts_np, support_np, meanings_np):
        return _fallback_numpy(utts_np, support_np, meanings_np)

    from concourse.bass_utils import run_bass_kernel_spmd

    nc = _get_nc()
    in_maps = _prep_in_maps(utts_np, support_np)
    t0 = time.monotonic_ns()
    res = run_bass_kernel_spmd(
        nc, in_maps, list(range(N_CORES)), trace=_trace, **_trace_kwargs
    )
    LAST_WALL_NS = time.monotonic_ns() - t0
    LAST_RESULTS = res

    vals = np.stack(
        [np.asarray(r["out"], dtype=np.float32) for r in res.results]
    )  # [8, 128, 16]: [core, p, t] -> query 128*t + p
    keys = np.rint(vals.transpose(0, 2, 1).reshape(N_CORES, N_QUERIES)).astype(
        np.int64
    )
    enc = 2 * ENC if _is_pd() else ENC
    s_local = np.mod(keys, enc)
    match = (s_local - keys) // enc  # match_count per core winner
    s_global = s_local + (np.arange(N_CORES, dtype=np.int64) * S_SHARD)[:, None]
    # global winner: max match_count, then smallest global index
    host_key = -match * (1 << 40) + s_global
    win = np.argmin(host_key, axis=0)
    idx = s_global[win, np.arange(N_QUERIES)]
    return _one_hot_meanings(meanings_np, idx)
